# revision 7
# baseline (speedup 1.0000x reference)
"""BuildCost kernel for Trainium2 (Bass/Tile), 8-core SPMD.

cost[b,d,p,q,c,uv] = xpad[b,c,u,v, p+off(d,u), q+off(d,v)] * mask[b,uv,p,q]
with off(d_idx,t) = d_idx*(4-t) + 4*t  (d_idx = d - mindisp), padded border 16.

Sharding: core = b*4 + qb  (b in {0,1}, qb in {0..3} -> q columns [qb*24, qb*24+24)).
Per core, one input blob [128, BF] holding (per partition):
  X   [(c,uv,qq=56)]    pre-padded window of x, pp on partitions
  M   [(uv,q=24)]       mask slice (valid on partitions 0..96 = p)
  IDN [(off=33, m=96)]  shifted identities (zero cols where row out of range)
Per (d,u): PSUM[p,(c',v,q)] = IDN[off(d,u)].T @ X[...]   (fp32 matmul, exact:
row shift via one-hot identity; col shift via rhs AP, v-stride = 60-d_idx).
Then OUT_d[p,(q,c,u,v)] = PSUM * mask (DVE/ACT, mask broadcast over c via
stride-0), and OUT_d -> HBM as one contiguous DMA per d.
"""

import numpy as np

from concourse import bacc, bass, mybir
from concourse.ap import AP
from concourse.tile import TileContext
from concourse.bass_utils import run_bass_kernel_spmd

A = 9            # angular resolution
D = 9            # disparities
H = W = 96
C = 4
PAD = 16
PPAD = 128       # padded row count
QB = W // 4      # q-block per core = 24
QQ = QB + 32     # stored qq window = 56
CUV = C * A * A  # 324
XF = C * A * A * QQ   # 18144  X region free size
MF = A * A * QB       # 1944   mask region free size
NOFF = 33
IDF = NOFF * H        # 3168   identity region free size
BF = XF + MF + IDF    # 23256  blob free size
OF = QB * CUV         # 7776   per-d output tile free size
F32 = mybir.dt.float32


def off(d_idx: int, t: int) -> int:
    return d_idx * (4 - t) + 4 * t


def _mkap(base: AP, extra_off: int, dims) -> AP:
    return AP(base.tensor, base.offset + extra_off, dims)


def build_body(nc: bass.Bass, tc, out: AP, blob: AP):
    with (
        tc.tile_pool(name="cst", bufs=1) as cst,
        tc.tile_pool(name="ob", bufs=2) as ob,
        tc.tile_pool(name="ps", bufs=8, space="PSUM") as ps,
    ):
        B = cst.tile([PPAD, BF], F32, tag="B")
        nc.sync.dma_start(B[:], blob)

        for d_idx in range(D):
            Od = ob.tile([H, OF], F32, tag="Od")
            for u in range(A):
                o_u = off(d_idx, u)
                lhsT = _mkap(B[:], XF + MF + o_u * H, [[BF, PPAD], [1, H]])
                for ch in range(2):
                    # rhs free iteration (c'(2), v(9), q(24)); v-stride folds the
                    # column shift off(d,v): 56 - (d_idx-4) = 60 - d_idx
                    rhs = _mkap(
                        B[:],
                        ch * 2 * (A * A * QQ) + u * A * QQ + 4 * d_idx,
                        [[BF, PPAD], [A * A * QQ, 2], [60 - d_idx, A], [1, QB]],
                    )
                    P = ps.tile([H, 432], F32, tag="ps")
                    nc.tensor.matmul(P[:], lhsT, rhs, start=True, stop=True)
                    # out iteration (q, c', v); psum layout (c', v, q); mask (uv, q)
                    oap = _mkap(
                        Od[:], u * A + ch * 2 * A * A,
                        [[OF, H], [CUV, QB], [A * A, 2], [1, A]],
                    )
                    pap = _mkap(P[:], 0, [[432, H], [1, QB], [216, 2], [QB, A]])
                    map_ = _mkap(
                        B[:], XF + u * A * QB,
                        [[BF, H], [1, QB], [0, 2], [QB, A]],
                    )
                    nc.any.tensor_mul(oap, pap, map_)
            nc.sync.dma_start(out[d_idx], Od[:])


def build_nc() -> bass.Bass:
    nc = bacc.Bacc("TRN2", target_bir_lowering=False, debug=False)
    blob = nc.dram_tensor("blob", [PPAD, BF], F32, kind="ExternalInput")
    out = nc.dram_tensor("out", [D, H, OF], F32, kind="ExternalOutput")
    with TileContext(nc) as tc:
        build_body(nc, tc, out.ap(), blob.ap())
    nc.finalize()
    return nc


def prep_xs(xb: np.ndarray, qb: int) -> np.ndarray:
    """xb [C,81,96,96] -> [128, XF] SBUF image: xs[pp,(c,uv,qq)] = xpad[c,uv,pp,q0+qq]."""
    q0 = qb * QB
    xs = np.zeros((PPAD, C, A * A, QQ), np.float32)
    lo, hi = max(PAD, q0), min(PAD + W, q0 + QQ)
    xs[PAD:PAD + H, :, :, lo - q0:hi - q0] = xb.transpose(2, 0, 1, 3)[:, :, :, lo - PAD:hi - PAD]
    return np.ascontiguousarray(xs.reshape(PPAD, XF))


def prep_msk(mb: np.ndarray, qb: int) -> np.ndarray:
    """mb [81,96,96] -> [128, MF] (rows 96..128 zero): m[p,(uv,q)] = mb[uv,p,q0+q]."""
    q0 = qb * QB
    m = np.zeros((PPAD, MF), np.float32)
    m[:H] = mb.transpose(1, 0, 2)[:, :, q0:q0 + QB].reshape(H, MF)
    return m


def prep_idn() -> np.ndarray:
    idn = np.zeros((PPAD, NOFF, H), np.float32)
    for o in range(NOFF):
        m = np.arange(H)
        k = m + o
        sel = (k >= PAD) & (k < PAD + H)
        idn[k[sel], o, m[sel]] = 1.0
    return np.ascontiguousarray(idn.reshape(PPAD, IDF))


def prep_blob(xb: np.ndarray, mb: np.ndarray, qb: int, idn: np.ndarray) -> np.ndarray:
    return np.ascontiguousarray(
        np.concatenate([prep_xs(xb, qb), prep_msk(mb, qb), idn], axis=1)
    )


_IDN = None


def kernel(x: np.ndarray, mask: np.ndarray):
    global _IDN
    x = np.asarray(x, np.float32)
    mask = np.asarray(mask, np.float32)
    ctr = x[:, :, 40:41, :, :].copy()
    if _IDN is None:
        _IDN = prep_idn()
    nc = build_nc()
    in_maps = []
    for core in range(8):
        b, qb = divmod(core, 4)
        in_maps.append({"blob": prep_blob(x[b], mask[b], qb, _IDN)})
    res = run_bass_kernel_spmd(nc, in_maps, list(range(8)))
    cost6 = np.empty((2, D, H, W, C, A * A), np.float32)
    for core in range(8):
        b, qb = divmod(core, 4)
        cost6[b, :, :, qb * QB:(qb + 1) * QB, :, :] = res.results[core]["out"].reshape(
            D, H, QB, C, A * A
        )
    cost = np.ascontiguousarray(cost6.reshape(2, D, H * W * C, A * A))
    return cost, ctr
